# revision 21
# baseline (speedup 1.0000x reference)
"""Trainium2 Bass kernel for nn_Attention_local (sparse routed attention).

Math (per batch b, head h):
  qkv = x @ Wqkv ; q,k,v per head (d=64)
  top-49 routing indices per (b,h,query) from adj logits
  attention over the selected 49 keys; gelu; @ Wv

Device strategy (8 cores, data-parallel over batch, 2 batches/core):
  - Replace the irregular gather with an exact threshold mask: per row,
    theta = 49th-largest of adj[b,h,i,:]; keep = adj >= theta selects
    exactly the top-49 set (validated: no ties at the boundary).
  - theta via 7x max8 + 6x match_replace rounds on DVE (fp32, exact).
  - s = (q*scale) @ k^T dense (bf16 matmul), e = exp(s) on ACT (|s|<0.5
    so no max-subtraction needed), ep = (adj>=theta)*e with fused row-sum
    on GPSIMD, normalize via gpsimd normalize_recip, transpose attn on PE,
    o^T = v^T-contract on PE, gelu on ACT, final projection on PE.
"""

import numpy as np
import ml_dtypes
from contextlib import ExitStack

import concourse.bass as bass
import concourse.tile as tile
from concourse import bacc, library_config, mybir
from concourse.bass_utils import run_bass_kernel_spmd

B, T, DIM = 16, 196, 512
H, D = 8, 64
TOPK = 49
NB = 2                 # batches per core
NPAIR = NB * H         # (b,h) pairs per core = 16
NCORES = 8
TA = 128               # query block A rows
TB = T - TA            # 68
NBF = 9                # flat selection tiles for B rows (16*68=1088 -> 9*128)
NBROWS = NPAIR * TB    # 1088
SCALE = DIM ** -0.5
BF = ml_dtypes.bfloat16
AF = mybir.ActivationFunctionType
ALU = mybir.AluOpType

# wave w handles pairs 4w..4w+3; B-flat tile bounds per wave (ceil(272(w+1)/128))
UB = [0, 3, 5, 7, 9]
NEG = -1.0e30

_PROGRAM_CACHE = {}


def _build_program(gelu=True):
    f32, bf16 = mybir.dt.float32, mybir.dt.bfloat16
    nc = bacc.Bacc("TRN2", target_bir_lowering=False, debug=False,
                   num_devices=NCORES)

    xT_d = nc.dram_tensor("xT", [4, 128, NB * T], bf16, kind="ExternalInput")
    wqk_d = nc.dram_tensor("wqk", [4, 128, 2 * DIM], bf16, kind="ExternalInput")
    wvp_d = nc.dram_tensor("wvp", [4, 128, DIM], bf16, kind="ExternalInput")
    wo_d = nc.dram_tensor("wo", [4, 128, DIM], bf16, kind="ExternalInput")
    adjA_d = nc.dram_tensor("adjA", [NPAIR, TA, T], f32, kind="ExternalInput")
    adjB_d = nc.dram_tensor("adjB", [NPAIR, TB, T], f32, kind="ExternalInput")
    adjBf_d = nc.dram_tensor("adjBf", [NBF, 128, T], f32, kind="ExternalInput")
    id_d = nc.dram_tensor("ident", [128, 128], bf16, kind="ExternalInput")
    out_d = nc.dram_tensor("out", [NB * T, DIM], f32, kind="ExternalOutput")

    with ExitStack() as ctx:
        tc = ctx.enter_context(tile.TileContext(nc))
        const = ctx.enter_context(tc.tile_pool(name="const", bufs=1))
        dram = ctx.enter_context(tc.tile_pool(name="dram", bufs=1, space="DRAM"))
        mx = ctx.enter_context(tc.tile_pool(name="mx", bufs=4))
        rsp = ctx.enter_context(tc.tile_pool(name="rsp", bufs=8))
        esb = ctx.enter_context(tc.tile_pool(name="esb", bufs=32))
        epsb = ctx.enter_context(tc.tile_pool(name="epsb", bufs=4))
        atsb = ctx.enter_context(tc.tile_pool(name="atsb", bufs=4))
        jsb = ctx.enter_context(tc.tile_pool(name="jsb", bufs=3))
        ps_mm = ctx.enter_context(tc.tile_pool(name="ps_mm", bufs=1, space="PSUM"))
        ps_s = ctx.enter_context(tc.tile_pool(name="ps_s", bufs=3, space="PSUM"))
        ps_j = ctx.enter_context(tc.tile_pool(name="ps_j", bufs=2, space="PSUM"))
        ps_o = ctx.enter_context(tc.tile_pool(name="ps_o", bufs=1, space="PSUM"))
        ps_f = ctx.enter_context(tc.tile_pool(name="ps_f", bufs=1, space="PSUM"))

        nc.gpsimd.load_library(library_config.attn)

        # ---------------- small constant loads first (unblock PE) ----------
        xT_sb = [const.tile([128, NB * T], bf16, name=f"xT{kc}", tag=f"xT{kc}") for kc in range(4)]
        wqk_sb = [const.tile([128, 2 * DIM], bf16, name=f"wqk{kc}", tag=f"wqk{kc}") for kc in range(4)]
        wvp_sb = [const.tile([128, DIM], bf16, name=f"wvp{kc}", tag=f"wvp{kc}") for kc in range(4)]
        wo_sb = [const.tile([128, DIM], bf16, name=f"wo{kc}", tag=f"wo{kc}") for kc in range(4)]
        ident = const.tile([128, 128], bf16)
        nc.sync.dma_start(ident[:], id_d[:])
        for kc in range(4):
            nc.sync.dma_start(xT_sb[kc][:], xT_d[kc])
            nc.sync.dma_start(wqk_sb[kc][:], wqk_d[kc])
            nc.sync.dma_start(wvp_sb[kc][:], wvp_d[kc])
            nc.sync.dma_start(wo_sb[kc][:], wo_d[kc])

        adjA_sb = const.tile([TA, NPAIR * T], f32)      # mask + selection source A
        adjB_sb = const.tile([TB, NPAIR * T], f32)      # mask compare, block B
        adjBf_sb = const.tile([128, NBF * T], f32)      # selection source B (flat)

        thA = const.tile([TA, NPAIR], f32)
        thB = const.tile([TB, NPAIR], f32)
        thBsel = const.tile([128, NBF], f32)
        thbB = dram.tile([NBF * 128], f32)

        # ---------------- q/k projection (PE): qT,kT head-major ----------------
        # wqk columns: [q of all heads (512) | k of all heads (512)], q pre-scaled.
        qkT_sb = [const.tile([D, NB * T], bf16, name=f"qkT{m}", tag=f"qkT{m}") for m in range(16)]
        # slots: 0..7 qT of head m ; 8..15 kT of head m-8
        for mt in range(8):
            ps = ps_mm.tile([128, NB * T], f32, name="qkps", tag="mm")
            for kc in range(4):
                nc.tensor.matmul(
                    ps[:], wqk_sb[kc][:, mt * 128:(mt + 1) * 128], xT_sb[kc][:],
                    start=(kc == 0), stop=(kc == 3))
            h0 = 2 * mt
            nc.scalar.activation(qkT_sb[h0][:], ps[0:D, :], AF.Copy)
            nc.scalar.activation(qkT_sb[h0 + 1][:], ps[D:128, :], AF.Copy)

        # ---------------- v projection (PE): v natural [token, DIM] ------------
        vA_sb = [const.tile([TA, DIM], bf16, name=f"vA{bi}", tag=f"vA{bi}") for bi in range(NB)]
        vB_sb = [const.tile([TB, DIM], bf16, name=f"vB{bi}", tag=f"vB{bi}") for bi in range(NB)]
        for bi in range(NB):
            psA = ps_mm.tile([TA, DIM], f32, name="vpsA", tag="mm")
            psB = ps_mm.tile([TB, DIM], f32, name="vpsB", tag="mm")
            for kc in range(4):
                c0 = bi * T
                nc.tensor.matmul(psA[:], xT_sb[kc][:, c0:c0 + TA], wvp_sb[kc][:],
                                 start=(kc == 0), stop=(kc == 3))
            for kc in range(4):
                c0 = bi * T + TA
                nc.tensor.matmul(psB[:], xT_sb[kc][:, c0:c0 + TB], wvp_sb[kc][:],
                                 start=(kc == 0), stop=(kc == 3))
            nc.scalar.activation(vA_sb[bi][:], psA[:], AF.Copy)
            nc.scalar.activation(vB_sb[bi][:], psB[:], AF.Copy)

        # ---------------- selection (DVE) ----------------
        wkp = ctx.enter_context(tc.tile_pool(name="wkp", bufs=3))

        def select49(src_seg, th_out):
            """th_out[:,0:1] <- 49th largest per row (src_seg left intact)."""
            m = mx.tile([src_seg.shape[0], 8], f32, name="m8", tag="m8")
            nc.vector.max(m[:], src_seg)
            seg = wkp.tile([128, T], f32, name="selwk", tag="selwk")
            seg = seg[0:src_seg.shape[0], :]
            nc.vector.match_replace(seg, m[:], src_seg, NEG)
            m = mx.tile([src_seg.shape[0], 8], f32, name="m8", tag="m8")
            nc.vector.max(m[:], seg)
            for _ in range(5):
                nc.vector.match_replace(seg, m[:], seg, NEG)
                m = mx.tile([src_seg.shape[0], 8], f32, name="m8", tag="m8")
                nc.vector.max(m[:], seg)
            nc.vector.tensor_copy(th_out, m[:, 0:1])

        # oT staging (f32, same layout as gT) so gelu runs as one batched
        # sweep at the end -- avoids per-pair Exp<->Gelu ACT table reloads.
        oT_sb = [const.tile([128, NB * T], f32, name=f"oT{kc}", tag=f"oT{kc}") for kc in range(4)]
        gT_sb = [const.tile([128, NB * T], bf16, name=f"gT{kc}", tag=f"gT{kc}") for kc in range(4)]

        def sel_b_wave(w):
            # B-row selection for wave w + DRAM bounce into per-pair layout
            for u in range(UB[w], UB[w + 1]):
                select49(adjBf_sb[:, u * T:(u + 1) * T], thBsel[:, u:u + 1])
            u0, u1 = UB[w], UB[w + 1]
            dst = thbB[:].rearrange("(u q) -> q u", q=128)[:, u0:u1]
            nc.sync.dma_start(dst, thBsel[:, u0:u1])
            srcv = thbB[0:NBROWS].rearrange("(p i) -> i p", p=NPAIR)
            nc.sync.dma_start(thB[:, 4 * w:4 * w + 4], srcv[:, 4 * w:4 * w + 4])

        # all adj input DMAs up front (wave order)
        for w in range(4):
            pair_rng = range(4 * w, 4 * w + 4)
            for u in range(UB[w], UB[w + 1]):
                nc.scalar.dma_start(adjBf_sb[:, u * T:(u + 1) * T], adjBf_d[u])
            for p in pair_rng:
                nc.scalar.dma_start(adjA_sb[:, p * T:(p + 1) * T], adjA_d[p])
            for p in pair_rng:
                nc.sync.dma_start(adjB_sb[:, p * T:(p + 1) * T], adjB_d[p])

        e_tiles = {}

        def s_exp_wave(w):
            # score matmuls + exp for a whole wave (emitted one wave ahead so
            # the in-order PE/ACT streams stay ahead of the DVE ep ops)
            for p in range(4 * w, 4 * w + 4):
                bi, hh = divmod(p, H)
                qT = qkT_sb[hh]
                kTs = qkT_sb[8 + hh][:, bi * T:bi * T + T]
                for blk, (P0, PN) in enumerate([(0, TA), (TA, TB)]):
                    s_ps = ps_s.tile([PN, T], f32, name="sps", tag="s")
                    nc.tensor.matmul(s_ps[:],
                                     qT[:, bi * T + P0:bi * T + P0 + PN], kTs,
                                     start=True, stop=True)
                    e_sb = esb.tile([PN, T], f32, name="et", tag="e")
                    nc.scalar.activation(e_sb[:], s_ps[:], AF.Exp)
                    e_tiles[(p, blk)] = e_sb

        # all selection first: DVE runs gap-free (selection only needs DMAs);
        # s+exp stream alongside on PE/ACT; attention pipelines at the end.
        for w in range(4):
            sel_b_wave(w)
            s_exp_wave(w)
            for p in range(4 * w, 4 * w + 4):
                select49(adjA_sb[:, p * T:(p + 1) * T], thA[:, p:p + 1])

        for w in range(4):
            # attention for this wave
            pair_rng = range(4 * w, 4 * w + 4)
            for p in pair_rng:
                bi, hh = divmod(p, H)
                qT = qkT_sb[hh]
                kT = qkT_sb[8 + hh]
                c0 = bi * T
                kTs = kT[:, c0:c0 + T]

                j_ps = ps_j.tile([128, 2 * T], mybir.dt.bfloat16, name="jps", tag="j")
                jA_ps = j_ps[:, 0:T]
                jB_ps = j_ps[0:TB, T:2 * T]

                for blk, (P0, PN, adj_sb, th) in enumerate(
                        [(0, TA, adjA_sb, thA), (TA, TB, adjB_sb, thB)]):
                    e_sb = e_tiles.pop((p, blk))
                    ep_sb = epsb.tile([PN, T], f32, name="ept", tag="ep")
                    rs_t = rsp.tile([PN, 1], f32, name="rst", tag=f"rs{blk}")
                    nc.vector.scalar_tensor_tensor(
                        ep_sb[:], adj_sb[:, p * T:(p + 1) * T], th[:, p:p + 1],
                        e_sb[:], op0=ALU.is_ge, op1=ALU.mult,
                        accum_out=rs_t[:])
                    at_sb = atsb.tile([PN, T], mybir.dt.bfloat16, name="att", tag="at")
                    nc.gpsimd.normalize_recip(at_sb[:], ep_sb[:], rs_t[:])
                    nc.tensor.transpose(
                        jA_ps[:, P0:P0 + PN], at_sb[:, 0:TA], ident[0:PN, 0:PN])
                    nc.tensor.transpose(
                        jB_ps[:, P0:P0 + PN], at_sb[:, TA:T], ident[0:PN, 0:PN])

                jA_sb = jsb.tile([TA, T], mybir.dt.bfloat16, name="jAsb", tag="jAs")
                jB_sb = jsb.tile([TB, T], mybir.dt.bfloat16, name="jBsb", tag="jBs")
                nc.scalar.activation(jA_sb[:], jA_ps[:], AF.Copy)
                nc.scalar.activation(jB_sb[:], jB_ps[:], AF.Copy)

                oT_ps = ps_o.tile([D, T], f32, name="oTps", tag="oT")
                nc.tensor.matmul(oT_ps[:], vA_sb[bi][:, hh * D:(hh + 1) * D],
                                 jA_sb[:], start=True, stop=False)
                nc.tensor.matmul(oT_ps[:], vB_sb[bi][:, hh * D:(hh + 1) * D],
                                 jB_sb[:], start=False, stop=True)
                ot = oT_sb[hh // 2]
                r0 = (hh % 2) * D
                nc.scalar.activation(ot[r0:r0 + D, c0:c0 + T], oT_ps[:], AF.Copy)

            # per-batch gelu + final projection as soon as a batch completes
            if w in (1, 3):
                bi = w // 2
                cb = bi * T
                for kc in range(4):
                    nc.scalar.activation(gT_sb[kc][:, cb:cb + T],
                                         oT_sb[kc][:, cb:cb + T],
                                         AF.Gelu if gelu else AF.Copy)
                for (P0, PN) in [(0, TA), (TA, TB)]:
                    ps = ps_f.tile([PN, DIM], f32, name="finps", tag="fin")
                    for kc in range(4):
                        nc.tensor.matmul(ps[:], gT_sb[kc][:, cb + P0:cb + P0 + PN],
                                         wo_sb[kc][:], start=(kc == 0), stop=(kc == 3))
                    o_sb = jsb.tile([PN, DIM], f32, name="osb", tag="osb")
                    nc.scalar.activation(o_sb[:], ps[:], AF.Copy)
                    nc.sync.dma_start(out_d[cb + P0: cb + P0 + PN, :], o_sb[:])


    nc.compile()
    return nc


def _prep_inputs(x, adj, Wqkv, Wv):
    """Host-side layout prep. Returns per-core in_maps."""
    x = np.asarray(x, np.float32)
    adj = np.asarray(adj, np.float32)
    Wqkv = np.asarray(Wqkv, np.float32)
    Wv = np.asarray(Wv, np.float32)

    # head-major re-pack of Wqkv columns: [q all heads | k all heads], v separate
    Wh = Wqkv.reshape(DIM, H, 3 * D)
    wq = np.concatenate([Wh[:, hh, 0:D] for hh in range(H)], axis=1) * SCALE
    wk = np.concatenate([Wh[:, hh, D:2 * D] for hh in range(H)], axis=1)
    wv = np.concatenate([Wh[:, hh, 2 * D:3 * D] for hh in range(H)], axis=1)
    wqk = np.concatenate([wq, wk], axis=1)                    # [512, 1024]
    wqk_t = wqk.reshape(4, 128, 2 * DIM).astype(BF)
    wvp_t = wv.reshape(4, 128, DIM).astype(BF)
    wo_t = Wv.reshape(4, 128, DIM).astype(BF)
    ident = np.eye(128, dtype=BF)

    in_maps = []
    for c in range(NCORES):
        xs = x[c * NB:(c + 1) * NB]                           # [2,196,512]
        xT = xs.transpose(2, 0, 1).reshape(DIM, NB * T)       # [512, 392]
        xT_t = xT.reshape(4, 128, NB * T).astype(BF)

        adj_c = adj[c * NB:(c + 1) * NB].reshape(NPAIR, T, T)  # pair-major
        adjA = np.ascontiguousarray(adj_c[:, 0:TA, :])
        adjB = np.ascontiguousarray(adj_c[:, TA:T, :])
        # flat-packed B rows for selection: row rb = p*68 + (i-128)
        brows = adj_c[:, TA:T, :].reshape(NBROWS, T)
        bpad = np.zeros((NBF * 128, T), np.float32)
        bpad[:NBROWS] = brows
        adjBf = np.ascontiguousarray(bpad.reshape(NBF, 128, T))

        in_maps.append({
            "xT": xT_t, "wqk": wqk_t, "wvp": wvp_t, "wo": wo_t,
            "adjA": adjA, "adjB": adjB, "adjBf": adjBf, "ident": ident,
        })
    return in_maps


def kernel(x, adj, Wqkv, Wv, topk, _trace=False):
    assert int(topk) == TOPK
    in_maps = _prep_inputs(x, adj, Wqkv, Wv)
    if "nc" not in _PROGRAM_CACHE:
        _PROGRAM_CACHE["nc"] = _build_program()
    nc = _PROGRAM_CACHE["nc"]
    res = run_bass_kernel_spmd(nc, in_maps, core_ids=list(range(NCORES)),
                               trace=_trace)
    out = np.empty((B, T, DIM), np.float32)
    for c in range(NCORES):
        out[c * NB:(c + 1) * NB] = res.results[c]["out"].reshape(NB, T, DIM)
    kernel._last_results = res
    return out


# revision 22
# speedup vs baseline: 1.0875x; 1.0875x over previous
"""Trainium2 Bass kernel for nn_Attention_local (sparse routed attention).

Math (per batch b, head h):
  qkv = x @ Wqkv ; q,k,v per head (d=64)
  top-49 routing indices per (b,h,query) from adj logits
  attention over the selected 49 keys; gelu; @ Wv

Device strategy (8 cores, data-parallel over batch, 2 batches/core):
  - Replace the irregular gather with an exact threshold mask: per row,
    theta = 49th-largest of adj[b,h,i,:]; keep = adj >= theta selects
    exactly the top-49 set (validated: no ties at the boundary).
  - theta via 7x max8 + 6x match_replace rounds on DVE (fp32, exact).
  - s = (q*scale) @ k^T dense (bf16 matmul), e = exp(s) on ACT (|s|<0.5
    so no max-subtraction needed), ep = (adj>=theta)*e with fused row-sum
    on GPSIMD, normalize via gpsimd normalize_recip, transpose attn on PE,
    o^T = v^T-contract on PE, gelu on ACT, final projection on PE.
"""

import numpy as np
import ml_dtypes
from contextlib import ExitStack

import concourse.bass as bass
import concourse.tile as tile
from concourse.tile import add_dep_helper
from concourse import bacc, library_config, mybir
from concourse.bass_utils import run_bass_kernel_spmd

B, T, DIM = 16, 196, 512
H, D = 8, 64
TOPK = 49
NB = 2                 # batches per core
NPAIR = NB * H         # (b,h) pairs per core = 16
NCORES = 8
TA = 128               # query block A rows
TB = T - TA            # 68
NBF = 9                # flat selection tiles for B rows (16*68=1088 -> 9*128)
NBROWS = NPAIR * TB    # 1088
SCALE = DIM ** -0.5
BF = ml_dtypes.bfloat16
AF = mybir.ActivationFunctionType
ALU = mybir.AluOpType

# wave w handles pairs 4w..4w+3; B-flat tile bounds per wave (ceil(272(w+1)/128))
UB = [0, 3, 5, 7, 9]
NEG = -1.0e30

_PROGRAM_CACHE = {}


def _build_program(gelu=True):
    f32, bf16 = mybir.dt.float32, mybir.dt.bfloat16
    nc = bacc.Bacc("TRN2", target_bir_lowering=False, debug=False,
                   num_devices=NCORES)

    xT_d = nc.dram_tensor("xT", [4, 128, NB * T], bf16, kind="ExternalInput")
    wqk_d = nc.dram_tensor("wqk", [4, 128, 2 * DIM], bf16, kind="ExternalInput")
    wvp_d = nc.dram_tensor("wvp", [4, 128, DIM], bf16, kind="ExternalInput")
    wo_d = nc.dram_tensor("wo", [4, 128, DIM], bf16, kind="ExternalInput")
    adjA_d = nc.dram_tensor("adjA", [NPAIR, TA, T], f32, kind="ExternalInput")
    adjB_d = nc.dram_tensor("adjB", [NPAIR, TB, T], f32, kind="ExternalInput")
    adjBf_d = nc.dram_tensor("adjBf", [NBF, 128, T], f32, kind="ExternalInput")
    id_d = nc.dram_tensor("ident", [128, 128], bf16, kind="ExternalInput")
    out_d = nc.dram_tensor("out", [NB * T, DIM], f32, kind="ExternalOutput")

    with ExitStack() as ctx:
        tc = ctx.enter_context(tile.TileContext(nc))
        const = ctx.enter_context(tc.tile_pool(name="const", bufs=1))
        dram = ctx.enter_context(tc.tile_pool(name="dram", bufs=1, space="DRAM"))
        mx = ctx.enter_context(tc.tile_pool(name="mx", bufs=4))
        rsp = ctx.enter_context(tc.tile_pool(name="rsp", bufs=8))
        esb = ctx.enter_context(tc.tile_pool(name="esb", bufs=32))
        epsb = ctx.enter_context(tc.tile_pool(name="epsb", bufs=4))
        atsb = ctx.enter_context(tc.tile_pool(name="atsb", bufs=4))
        jsb = ctx.enter_context(tc.tile_pool(name="jsb", bufs=3))
        ps_mm = ctx.enter_context(tc.tile_pool(name="ps_mm", bufs=1, space="PSUM"))
        ps_s = ctx.enter_context(tc.tile_pool(name="ps_s", bufs=3, space="PSUM"))
        ps_j = ctx.enter_context(tc.tile_pool(name="ps_j", bufs=2, space="PSUM"))
        ps_o = ctx.enter_context(tc.tile_pool(name="ps_o", bufs=1, space="PSUM"))
        ps_f = ctx.enter_context(tc.tile_pool(name="ps_f", bufs=1, space="PSUM"))

        nc.gpsimd.load_library(library_config.attn)

        # ---------------- small constant loads first (unblock PE) ----------
        xT_sb = [const.tile([128, NB * T], bf16, name=f"xT{kc}", tag=f"xT{kc}") for kc in range(4)]
        wqk_sb = [const.tile([128, 2 * DIM], bf16, name=f"wqk{kc}", tag=f"wqk{kc}") for kc in range(4)]
        wvp_sb = [const.tile([128, DIM], bf16, name=f"wvp{kc}", tag=f"wvp{kc}") for kc in range(4)]
        wo_sb = [const.tile([128, DIM], bf16, name=f"wo{kc}", tag=f"wo{kc}") for kc in range(4)]
        ident = const.tile([128, 128], bf16)
        nc.sync.dma_start(ident[:], id_d[:])
        for kc in range(4):
            nc.sync.dma_start(xT_sb[kc][:], xT_d[kc])
            nc.sync.dma_start(wqk_sb[kc][:], wqk_d[kc])
            nc.sync.dma_start(wvp_sb[kc][:], wvp_d[kc])
            nc.sync.dma_start(wo_sb[kc][:], wo_d[kc])

        adjA_sb = const.tile([TA, NPAIR * T], f32)      # mask + selection source A
        adjB_sb = const.tile([TB, NPAIR * T], f32)      # mask compare, block B
        adjBf_sb = const.tile([128, NBF * T], f32)      # selection source B (flat)

        thA = const.tile([TA, NPAIR], f32)
        thB = const.tile([TB, NPAIR], f32)
        thBsel = const.tile([128, NBF], f32)
        thbB = dram.tile([NBF * 128], f32)

        # ---------------- q/k projection (PE): qT,kT head-major ----------------
        # wqk columns: [q of all heads (512) | k of all heads (512)], q pre-scaled.
        qkT_sb = [const.tile([D, NB * T], bf16, name=f"qkT{m}", tag=f"qkT{m}") for m in range(16)]
        # slots: 0..7 qT of head m ; 8..15 kT of head m-8
        for mt in range(8):
            ps = ps_mm.tile([128, NB * T], f32, name="qkps", tag="mm")
            for kc in range(4):
                nc.tensor.matmul(
                    ps[:], wqk_sb[kc][:, mt * 128:(mt + 1) * 128], xT_sb[kc][:],
                    start=(kc == 0), stop=(kc == 3))
            h0 = 2 * mt
            nc.scalar.activation(qkT_sb[h0][:], ps[0:D, :], AF.Copy)
            nc.scalar.activation(qkT_sb[h0 + 1][:], ps[D:128, :], AF.Copy)

        # ---------------- v projection (PE): v natural [token, DIM] ------------
        vA_sb = [const.tile([TA, DIM], bf16, name=f"vA{bi}", tag=f"vA{bi}") for bi in range(NB)]
        vB_sb = [const.tile([TB, DIM], bf16, name=f"vB{bi}", tag=f"vB{bi}") for bi in range(NB)]
        for bi in range(NB):
            psA = ps_mm.tile([TA, DIM], f32, name="vpsA", tag="mm")
            psB = ps_mm.tile([TB, DIM], f32, name="vpsB", tag="mm")
            for kc in range(4):
                c0 = bi * T
                nc.tensor.matmul(psA[:], xT_sb[kc][:, c0:c0 + TA], wvp_sb[kc][:],
                                 start=(kc == 0), stop=(kc == 3))
            for kc in range(4):
                c0 = bi * T + TA
                nc.tensor.matmul(psB[:], xT_sb[kc][:, c0:c0 + TB], wvp_sb[kc][:],
                                 start=(kc == 0), stop=(kc == 3))
            nc.scalar.activation(vA_sb[bi][:], psA[:], AF.Copy)
            nc.scalar.activation(vB_sb[bi][:], psB[:], AF.Copy)

        # ---------------- selection (DVE) ----------------
        wkp = ctx.enter_context(tc.tile_pool(name="wkp", bufs=3))

        def select49(src_seg, th_out):
            """th_out[:,0:1] <- 49th largest per row (src_seg left intact)."""
            m = mx.tile([src_seg.shape[0], 8], f32, name="m8", tag="m8")
            nc.vector.max(m[:], src_seg)
            seg = wkp.tile([128, T], f32, name="selwk", tag="selwk")
            seg = seg[0:src_seg.shape[0], :]
            nc.vector.match_replace(seg, m[:], src_seg, NEG)
            m = mx.tile([src_seg.shape[0], 8], f32, name="m8", tag="m8")
            nc.vector.max(m[:], seg)
            for _ in range(5):
                nc.vector.match_replace(seg, m[:], seg, NEG)
                m = mx.tile([src_seg.shape[0], 8], f32, name="m8", tag="m8")
                nc.vector.max(m[:], seg)
            return nc.vector.tensor_copy(th_out, m[:, 0:1])

        # oT staging (f32, same layout as gT) so gelu runs as one batched
        # sweep at the end -- avoids per-pair Exp<->Gelu ACT table reloads.
        oT_sb = [const.tile([128, NB * T], f32, name=f"oT{kc}", tag=f"oT{kc}") for kc in range(4)]
        gT_sb = [const.tile([128, NB * T], bf16, name=f"gT{kc}", tag=f"gT{kc}") for kc in range(4)]

        def sel_b_wave(w):
            # B-row selection for wave w + DRAM bounce into per-pair layout
            for u in range(UB[w], UB[w + 1]):
                select49(adjBf_sb[:, u * T:(u + 1) * T], thBsel[:, u:u + 1])
            u0, u1 = UB[w], UB[w + 1]
            dst = thbB[:].rearrange("(u q) -> q u", q=128)[:, u0:u1]
            nc.sync.dma_start(dst, thBsel[:, u0:u1])
            srcv = thbB[0:NBROWS].rearrange("(p i) -> i p", p=NPAIR)
            nc.sync.dma_start(thB[:, 4 * w:4 * w + 4], srcv[:, 4 * w:4 * w + 4])

        # all adj input DMAs up front (wave order)
        for w in range(4):
            pair_rng = range(4 * w, 4 * w + 4)
            for u in range(UB[w], UB[w + 1]):
                nc.scalar.dma_start(adjBf_sb[:, u * T:(u + 1) * T], adjBf_d[u])
            for p in pair_rng:
                nc.scalar.dma_start(adjA_sb[:, p * T:(p + 1) * T], adjA_d[p])
            for p in pair_rng:
                nc.sync.dma_start(adjB_sb[:, p * T:(p + 1) * T], adjB_d[p])

        e_tiles = {}

        def s_exp_wave(w):
            # score matmuls + exp for a whole wave (emitted one wave ahead so
            # the in-order PE/ACT streams stay ahead of the DVE ep ops)
            for p in range(4 * w, 4 * w + 4):
                bi, hh = divmod(p, H)
                qT = qkT_sb[hh]
                kTs = qkT_sb[8 + hh][:, bi * T:bi * T + T]
                for blk, (P0, PN) in enumerate([(0, TA), (TA, TB)]):
                    s_ps = ps_s.tile([PN, T], f32, name="sps", tag="s")
                    nc.tensor.matmul(s_ps[:],
                                     qT[:, bi * T + P0:bi * T + P0 + PN], kTs,
                                     start=True, stop=True)
                    e_sb = esb.tile([PN, T], f32, name="et", tag="e")
                    nc.scalar.activation(e_sb[:], s_ps[:], AF.Exp)
                    e_tiles[(p, blk)] = e_sb

        # all selection first: DVE runs gap-free (selection only needs DMAs);
        # s+exp stream alongside on PE/ACT; attention pipelines at the end.
        last_sel = None
        for w in range(4):
            sel_b_wave(w)
            s_exp_wave(w)
            for p in range(4 * w, 4 * w + 4):
                last_sel = select49(adjA_sb[:, p * T:(p + 1) * T],
                                    thA[:, p:p + 1])

        for w in range(4):
            # attention for this wave
            pair_rng = range(4 * w, 4 * w + 4)
            for p in pair_rng:
                bi, hh = divmod(p, H)
                qT = qkT_sb[hh]
                kT = qkT_sb[8 + hh]
                c0 = bi * T
                kTs = kT[:, c0:c0 + T]

                j_ps = ps_j.tile([128, 2 * T], mybir.dt.bfloat16, name="jps", tag="j")
                jA_ps = j_ps[:, 0:T]
                jB_ps = j_ps[0:TB, T:2 * T]

                for blk, (P0, PN, adj_sb, th) in enumerate(
                        [(0, TA, adjA_sb, thA), (TA, TB, adjB_sb, thB)]):
                    e_sb = e_tiles.pop((p, blk))
                    ep_sb = epsb.tile([PN, T], f32, name="ept", tag="ep")
                    rs_t = rsp.tile([PN, 1], f32, name="rst", tag=f"rs{blk}")
                    epi = nc.vector.scalar_tensor_tensor(
                        ep_sb[:], adj_sb[:, p * T:(p + 1) * T], th[:, p:p + 1],
                        e_sb[:], op0=ALU.is_ge, op1=ALU.mult,
                        accum_out=rs_t[:])
                    add_dep_helper(epi.ins, last_sel.ins, sync=True,
                                   reason="eps after all selection")
                    at_sb = atsb.tile([PN, T], mybir.dt.bfloat16, name="att", tag="at")
                    nc.gpsimd.normalize_recip(at_sb[:], ep_sb[:], rs_t[:])
                    nc.tensor.transpose(
                        jA_ps[:, P0:P0 + PN], at_sb[:, 0:TA], ident[0:PN, 0:PN])
                    nc.tensor.transpose(
                        jB_ps[:, P0:P0 + PN], at_sb[:, TA:T], ident[0:PN, 0:PN])

                jA_sb = jsb.tile([TA, T], mybir.dt.bfloat16, name="jAsb", tag="jAs")
                jB_sb = jsb.tile([TB, T], mybir.dt.bfloat16, name="jBsb", tag="jBs")
                nc.scalar.activation(jA_sb[:], jA_ps[:], AF.Copy)
                nc.scalar.activation(jB_sb[:], jB_ps[:], AF.Copy)

                oT_ps = ps_o.tile([D, T], f32, name="oTps", tag="oT")
                nc.tensor.matmul(oT_ps[:], vA_sb[bi][:, hh * D:(hh + 1) * D],
                                 jA_sb[:], start=True, stop=False)
                nc.tensor.matmul(oT_ps[:], vB_sb[bi][:, hh * D:(hh + 1) * D],
                                 jB_sb[:], start=False, stop=True)
                ot = oT_sb[hh // 2]
                r0 = (hh % 2) * D
                nc.scalar.activation(ot[r0:r0 + D, c0:c0 + T], oT_ps[:], AF.Copy)

            # per-batch gelu + final projection as soon as a batch completes
            if w in (1, 3):
                bi = w // 2
                cb = bi * T
                for kc in range(4):
                    nc.scalar.activation(gT_sb[kc][:, cb:cb + T],
                                         oT_sb[kc][:, cb:cb + T],
                                         AF.Gelu if gelu else AF.Copy)
                for (P0, PN) in [(0, TA), (TA, TB)]:
                    ps = ps_f.tile([PN, DIM], f32, name="finps", tag="fin")
                    for kc in range(4):
                        nc.tensor.matmul(ps[:], gT_sb[kc][:, cb + P0:cb + P0 + PN],
                                         wo_sb[kc][:], start=(kc == 0), stop=(kc == 3))
                    o_sb = jsb.tile([PN, DIM], f32, name="osb", tag="osb")
                    nc.scalar.activation(o_sb[:], ps[:], AF.Copy)
                    nc.sync.dma_start(out_d[cb + P0: cb + P0 + PN, :], o_sb[:])


    nc.compile()
    return nc


def _prep_inputs(x, adj, Wqkv, Wv):
    """Host-side layout prep. Returns per-core in_maps."""
    x = np.asarray(x, np.float32)
    adj = np.asarray(adj, np.float32)
    Wqkv = np.asarray(Wqkv, np.float32)
    Wv = np.asarray(Wv, np.float32)

    # head-major re-pack of Wqkv columns: [q all heads | k all heads], v separate
    Wh = Wqkv.reshape(DIM, H, 3 * D)
    wq = np.concatenate([Wh[:, hh, 0:D] for hh in range(H)], axis=1) * SCALE
    wk = np.concatenate([Wh[:, hh, D:2 * D] for hh in range(H)], axis=1)
    wv = np.concatenate([Wh[:, hh, 2 * D:3 * D] for hh in range(H)], axis=1)
    wqk = np.concatenate([wq, wk], axis=1)                    # [512, 1024]
    wqk_t = wqk.reshape(4, 128, 2 * DIM).astype(BF)
    wvp_t = wv.reshape(4, 128, DIM).astype(BF)
    wo_t = Wv.reshape(4, 128, DIM).astype(BF)
    ident = np.eye(128, dtype=BF)

    in_maps = []
    for c in range(NCORES):
        xs = x[c * NB:(c + 1) * NB]                           # [2,196,512]
        xT = xs.transpose(2, 0, 1).reshape(DIM, NB * T)       # [512, 392]
        xT_t = xT.reshape(4, 128, NB * T).astype(BF)

        adj_c = adj[c * NB:(c + 1) * NB].reshape(NPAIR, T, T)  # pair-major
        adjA = np.ascontiguousarray(adj_c[:, 0:TA, :])
        adjB = np.ascontiguousarray(adj_c[:, TA:T, :])
        # flat-packed B rows for selection: row rb = p*68 + (i-128)
        brows = adj_c[:, TA:T, :].reshape(NBROWS, T)
        bpad = np.zeros((NBF * 128, T), np.float32)
        bpad[:NBROWS] = brows
        adjBf = np.ascontiguousarray(bpad.reshape(NBF, 128, T))

        in_maps.append({
            "xT": xT_t, "wqk": wqk_t, "wvp": wvp_t, "wo": wo_t,
            "adjA": adjA, "adjB": adjB, "adjBf": adjBf, "ident": ident,
        })
    return in_maps


def kernel(x, adj, Wqkv, Wv, topk, _trace=False):
    assert int(topk) == TOPK
    in_maps = _prep_inputs(x, adj, Wqkv, Wv)
    if "nc" not in _PROGRAM_CACHE:
        _PROGRAM_CACHE["nc"] = _build_program()
    nc = _PROGRAM_CACHE["nc"]
    res = run_bass_kernel_spmd(nc, in_maps, core_ids=list(range(NCORES)),
                               trace=_trace)
    out = np.empty((B, T, DIM), np.float32)
    for c in range(NCORES):
        out[c * NB:(c + 1) * NB] = res.results[c]["out"].reshape(NB, T, DIM)
    kernel._last_results = res
    return out
